# revision 8
# baseline (speedup 1.0000x reference)
"""BayesianPDA Trainium2 kernel, v2: software-pipelined half-chains.

DP (per row s): mu'[t] = max(mu[t], mu[t-1]) + w[t] + ln(1+exp(-|d|)),
d2[t] = mu[t-1] - mu[t] stored for the output sigmoid.
Output: pi0 = sigmoid(-d2) = 1/(1+e^{d2}), pi1 = e^{d2}/(1+e^{d2}).

The serial chain is ACT-latency-bound (EXP+LN ~391ns each, 2/row).  Two
independent 64-partition half-chains (2 batches each) are software-
pipelined so ACT runs expA,lnA,expB,lnB back-to-back (~1564ns / 2 rows)
while DVE covers the other half's element ops in the shadow.

Layout per core: 128 partitions = 4 batches x 32 column-groups; group g
holds DP columns j=32g+1..32g+32 at tile slots t=8..39 (j = 32g+t-7),
8-col left halo t=0..7 refreshed every 8 steps per half with a TensorE
shift-matmul + DVE PSUM-copy (bcol re-installs the group-0 sentinel).

d2 rows go to a 128-row ring (40-float stride, 32B-aligned slices); every
RS rows a sigmoid chunk is interleaved into the DP using exp (same ACT
table as the DP - no table switch) + fast DVE reciprocal:
    pi0 = r = 1/(1+e^{min(d2,30)}), pi1 = e*r
then DMAed out.  Saturated tails are fine: |err| < 1e-13 and the harness
rel-err denominator floors at 1e-6.
"""
import sys, os
sys.path.insert(0, '/opt/trn_rl_repo')
from contextlib import ExitStack

import numpy as np

import concourse.bass as bass
import concourse.tile as tile
from concourse import bacc, mybir
from concourse.bass_utils import run_bass_kernel_spmd

AF = mybir.ActivationFunctionType
ALU = mybir.AluOpType
F32 = mybir.dt.float32

# The act-table-load placer walks the table list in order and loads the first
# set containing the requested function.  Exp and Ln both live in
# "natural_log_exp_and_others"; hide them from every other set so the placer
# always lands on the joint one (dict order must stay untouched: the emitted
# act_func_set_id is positional against act_info.json).
_orig_get_tables = bacc.get_activation_tables


def _patched_get_tables(arch):
    t = _orig_get_tables(arch)
    pref = "natural_log_exp_and_others"
    if pref in t:
        out = {}
        for k, v in t.items():
            if k != pref:
                v = {f for f in v if f not in (AF.Exp, AF.Ln)}
            out[k] = v
        t = out
    return t


bacc.get_activation_tables = _patched_get_tables

B, NA, NB = 32, 1024, 1024
NCORES = 8
BPC = B // NCORES              # batches per core
G = 32                         # column groups
HALO = 16
T = 32 + HALO                  # mu tile width (40)
DW = T - 1                     # meaningful d width (39)
OWN0 = HALO - 1                # d slot of the first owned column
NEG = -1e20
CH = 64                        # W-chunk: DP steps per DMA chunk
RS = 64                        # sigmoid rows per interleaved chunk
RING = 128                     # d2 ring rows
SIG_LAG = 8                    # inject chunk c at s = c*RS + RS + SIG_LAG
CLAMP = 30.0
DWS = 48                       # d ring stride (192B: every slice 64B-aligned)
MUOFF = 15                     # mu offset in mubig: write mu[1:T] at byte 64
# Single 128-partition chain: splitting partitions into pipelined half-
# chains doubles the ACT op count per DP row (4x ~330ns, measured 2.02ms
# total) and ACT saturates above the single-chain latency bound (~1.3ms).
HV = ((0, 128),)


def _build(ns: int):
    """ns = number of DP rows (NA for the real kernel)."""
    nc = bacc.Bacc("TRN2", target_bir_lowering=False, debug=False,
                   num_devices=NCORES)
    w_d = nc.dram_tensor("W", [BPC, NA, NB], F32, kind="ExternalInput")
    s1_d = nc.dram_tensor("S1B", [128, 128], F32, kind="ExternalInput")
    bc_d = nc.dram_tensor("BCOL", [128, 8], F32, kind="ExternalInput")
    out_d = nc.dram_tensor("out", [BPC, ns, NB, 2], F32, kind="ExternalOutput")

    n_ch = (ns + CH - 1) // CH

    with tile.TileContext(nc) as tc, ExitStack() as ctx:
        consts = ctx.enter_context(tc.tile_pool(name="consts", bufs=1))
        state = ctx.enter_context(tc.tile_pool(name="state", bufs=1))
        ringp = ctx.enter_context(tc.tile_pool(name="ring", bufs=1))
        wpool = ctx.enter_context(tc.tile_pool(name="wpool", bufs=1))
        scratch = ctx.enter_context(tc.tile_pool(name="scratch", bufs=3))
        sig = ctx.enter_context(tc.tile_pool(name="sig", bufs=2))
        stage = ctx.enter_context(tc.tile_pool(name="stage", bufs=2))
        psum = ctx.enter_context(
            tc.tile_pool(name="psum", bufs=2, space="PSUM"))

        s1 = consts.tile([128, 128], F32)
        nc.sync.dma_start(s1[:], s1_d.ap())
        bcol = consts.tile([128, 8], F32)
        nc.sync.dma_start(bcol[:], bc_d.ap())

        # mu lives at mubig cols 7..46 so the hot write mu[:,1:T] starts at
        # byte 32 of the partition row (32B-aligned).
        mubig = state.tile([128, 64], F32)
        nc.vector.memset(mubig[:], NEG)

        def MU(p0, p1, a, b):
            return mubig[p0:p1, MUOFF + a:MUOFF + b]

        # mu[row 0, j=0] = 0 lives at t=HALO-1 of group 0 of each batch
        for b in range(BPC):
            nc.vector.memset(
                mubig[32 * b:32 * b + 1, MUOFF + HALO - 1:MUOFF + HALO], 0.0)

        dring = ringp.tile([128, RING * DWS], F32)

        # Two persistent W staging buffers (ping-pong).  Slot (s_local, k),
        # k=0..DW-1 holds w for tile position t=k+1: W[b, s-1, 32g + t - 8].
        wbufs = [wpool.tile([128, CH * DWS], F32, name=f"wbuf{i}",
                            tag=f"wbuf{i}") for i in range(2)]
        for wb in wbufs:
            # group-0 halo slots (t=1..HALO-1): absorbing -1e20 pad
            wb3 = wb[:, :].rearrange("p (s k) -> p s k", k=DWS)
            for b in range(BPC):
                nc.vector.memset(wb3[32 * b:32 * b + 1, :, 0:HALO - 1], NEG)

        def dma_w_chunk(ci: int):
            wb = wbufs[ci % 2]
            s0 = ci * CH
            rows = min(CH, ns - s0)
            wb3 = wb[:, :].rearrange("p (s k) -> p s k", k=DWS)
            for b in range(BPC):
                # owned columns t=HALO..T-1 (k=HALO-1..DW-1)
                src = w_d.ap()[b, s0:s0 + rows, :] \
                    .rearrange("s (g c) -> g s c", c=32)
                nc.sync.dma_start(
                    wb3[32 * b:32 * b + 32, 0:rows, HALO - 1:DW], src)
                # halo: groups 1..31, t=1..HALO-1, w cols 32g-HALO+1..32g-1
                srch = w_d.ap()[b, s0:s0 + rows,
                                32 - (HALO - 1):32 - (HALO - 1) + 31 * 32] \
                    .rearrange("s (g c) -> g s c", c=32)[:, :, 0:HALO - 1]
                nc.sync.dma_start(
                    wb3[32 * b + 1:32 * b + 32, 0:rows, 0:HALO - 1], srch)

        dma_w_chunk(0)

        def phase1(hi, s):
            """d2/nad/exp + (max,+w) for half hi, step s.  Returns (e,m)."""
            p0, p1 = HV[hi]
            ci = (s - 1) // CH
            sl = (s - 1) % CH
            wb3 = wbufs[ci % 2][:, :].rearrange("p (s k) -> p s k", k=DWS)
            w_s = wb3[p0:p1, sl, 0:DW]
            row = (s - 1) % RING
            dsl = dring[p0:p1, row * DWS:row * DWS + DW]
            # d2 = mu[t-1] - mu[t] (to scratch; ring copy runs in ACT shadow)
            d_s = scratch.tile([128, DWS], F32, tag=f"d{hi}",
                               name=f"d{hi}_{s}")[p0:p1, 0:DW]
            nc.vector.tensor_sub(d_s, MU(p0, p1, 0, T - 1), MU(p0, p1, 1, T))
            # nad = -|d2| = min(-d2, d2)
            nad = scratch.tile([128, DWS], F32, tag=f"nad{hi}",
                               name=f"nad{hi}_{s}")[p0:p1, 0:DW]
            nc.vector.scalar_tensor_tensor(nad, d_s, -1.0, d_s,
                                           ALU.mult, ALU.min)
            e_t = scratch.tile([128, DWS], F32, tag=f"e{hi}",
                               name=f"e{hi}_{s}")[p0:p1, 0:DW]
            nc.scalar.activation(e_t, nad, AF.Exp)
            # mw = max(mu[t], mu[t-1]) + w
            m_t = scratch.tile([128, DWS], F32, tag=f"m{hi}",
                               name=f"m{hi}_{s}")[p0:p1, 0:DW]
            nc.vector.tensor_max(m_t, MU(p0, p1, 1, T), MU(p0, p1, 0, T - 1))
            nc.vector.tensor_add(m_t, m_t, w_s)
            nc.vector.tensor_copy(dsl, d_s)
            return e_t, m_t

        def ln_op(hi, s, e_t):
            l_t = scratch.tile([128, DWS], F32, tag=f"l{hi}",
                               name=f"l{hi}_{s}")[HV[hi][0]:HV[hi][1], 0:DW]
            nc.scalar.activation(l_t, e_t, AF.Ln, bias=1.0)
            return l_t

        def add_op(hi, m_t, l_t):
            p0, p1 = HV[hi]
            nc.vector.tensor_add(MU(p0, p1, 1, T), m_t, l_t)

        def halo(hi):
            """tile_g[t=0..7] <- tile_{g-1}[t=32..39] for one half; the
            per-partition bcol re-installs -1e20 on group-0 rows (their
            shift-matrix column is all zero -> psum row is 0)."""
            p0, p1 = HV[hi]
            ph = psum.tile([128, HALO], F32, tag=f"ph{hi}",
                           name=f"ph{hi}")[p0:p1, :]
            nc.tensor.matmul(ph, s1[p0:p1, p0:p1], MU(p0, p1, 32, T),
                             start=True, stop=True)
            nc.vector.tensor_scalar(MU(p0, p1, 0, HALO), ph,
                                    bcol[p0:p1, 0:1], None, ALU.add)

        dr3 = dring[:, :].rearrange("p (s k) -> p s k", k=DWS)

        def sig_chunk(c):
            """pi rows [c*RS, c*RS+rows): exp (shared table) + fast recip."""
            i0 = c * RS
            rows = min(RS, ns - i0)
            slot = i0 % RING
            dv = dr3[:, slot:slot + rows, OWN0:OWN0 + 32]
            n = rows * 32
            dc = sig.tile([128, RS * 32], F32, tag="dcs")
            dc3 = dc[:, 0:n].rearrange("p (s c) -> p s c", c=32)
            nc.vector.tensor_scalar_min(dc3, dv, CLAMP)
            e_s = sig.tile([128, RS * 32], F32, tag="es")
            nc.scalar.activation(e_s[:, 0:n], dc[:, 0:n], AF.Exp)
            ep = sig.tile([128, RS * 32], F32, tag="ep")
            nc.vector.tensor_scalar_add(ep[:, 0:n], e_s[:, 0:n], 1.0)
            pi_st = stage.tile([128, RS * 64], F32, tag="pi")
            pi4 = pi_st[:, :].rearrange("p (s c k) -> p s c k", c=32, k=2)
            e3 = e_s[:, 0:n].rearrange("p (s c) -> p s c", c=32)
            ep3 = ep[:, 0:n].rearrange("p (s c) -> p s c", c=32)
            nc.vector.reciprocal_approx_fast(pi4[:, 0:rows, :, 0], ep3)
            nc.vector.tensor_mul(pi4[:, 0:rows, :, 1], e3,
                                 pi4[:, 0:rows, :, 0])
            for b in range(BPC):
                dst = out_d.ap()[b, i0:i0 + rows, :, :] \
                    .rearrange("s (g c) k -> g s (c k)", c=32)
                src = pi_st[32 * b:32 * b + 32, :] \
                    .rearrange("p (s ck) -> p s ck", ck=64)[:, 0:rows, :]
                nc.sync.dma_start(dst, src)

        n_sig = (ns + RS - 1) // RS
        sig_done = 0

        e_t, m_t = phase1(0, 1)
        for s in range(1, ns + 1):
            ci = (s - 1) // CH
            if (s - 1) % CH == 0 and ci + 1 < n_ch:
                dma_w_chunk(ci + 1)
            l_t = ln_op(0, s, e_t)
            add_op(0, m_t, l_t)
            if s % HALO == 0 and s < ns:
                halo(0)
            if (sig_done < n_sig - 1
                    and s == sig_done * RS + RS + SIG_LAG):
                sig_chunk(sig_done)
                sig_done += 1
            if s < ns:
                e_t, m_t = phase1(0, s + 1)

        while sig_done < n_sig:
            sig_chunk(sig_done)
            sig_done += 1

    nc.compile()
    return nc


def _consts():
    s1 = np.zeros((128, 128), dtype=np.float32)
    bcol = np.zeros((128, 8), dtype=np.float32)
    for p in range(128):
        if p % 32 == 0:
            bcol[p, 0] = np.float32(NEG)
        else:
            s1[p - 1, p] = 1.0
    return s1, bcol


_cache = {}


def _get_nc(ns: int):
    if ns not in _cache:
        _cache[ns] = _build(ns)
    return _cache[ns]


def kernel(W: np.ndarray, mask: np.ndarray, ns: int = NA, **run_kwargs):
    W = np.ascontiguousarray(W, dtype=np.float32)
    nc = _get_nc(ns)
    s1, bcol = _consts()
    in_maps = [{"W": W[c * BPC:(c + 1) * BPC], "S1B": s1, "BCOL": bcol}
               for c in range(NCORES)]
    res = run_bass_kernel_spmd(nc, in_maps, core_ids=list(range(NCORES)),
                               **run_kwargs)
    pi = np.concatenate([r["out"] for r in res.results], axis=0)
    kernel.last_result = res
    if not mask.all():
        pi = pi * np.asarray(mask)[:, :ns, :, None].astype(np.float32)
    return pi
